# revision 17
# baseline (speedup 1.0000x reference)
"""Causal attention (B=8, S=2048, D=H=768) on 8 trn2 NeuronCores.

Data-parallel over batch: core c computes batch c entirely on-chip, no
collectives.  All matmuls contract over the partition dim.

Key algebraic move: scores = (x Wq)(x Wk)^T = x (Wq Wk^T) x^T, with
M = Wq Wk^T precomputed on host (768x768, ~0.5 GFLOP — negligible).  That
folds the q AND k projections into ONE on-device projection t = x M, and
the scores' k-side operand becomes raw x^T, whose exact bf16 hi/lo splits
ship straight from the host.

Precision scheme (validated vs fp64 in numpy: ~4e-4 rel absmax):
  - t = x M and scores = t x^T use bf16 hi/lo splits with 3-term matmuls
    (hi*hi + hi*lo + lo*hi, fp32 PSUM accumulation) — ~17-bit effective
    mantissa at 3 PE cycles/row.
  - V projection, exp weights, transposes, and attn@V run in fp16
    (11-bit mantissa, 1 cycle/row, fast weight load).
  - softmax stats (rowmax, rowsum, reciprocal) in fp32.

Per-core pipeline:
  phase 1b (first — small DMAs get PE going fastest):
      V[s,h] = x^T-blocks (stationary) x Wv (moving), fp16
  phase 1a: tT = M (stationary) x xT (moving), split to bf16 hi/lo
  phase 2, per 128-row q-tile (descending, so the exposed tail chain is
    the smallest tile): scores strip [q, k<=q]; causal mask on diag block;
    rowmax; exp (ScalarE, bias=-rowmax, accum_out=rowsum) -> fp16;
    PE-transpose exp blocks -> expT [k,q]; out = sum_k expT x V; scale by
    1/rowsum.

Host side: shards x over batch, pre-transposes/splits, computes M,
replicates weights, gathers per-core outputs.
"""

from contextlib import ExitStack

import ml_dtypes
import numpy as np

import bass_rust
import concourse.bass as bass
import concourse.mybir as mybir
import concourse.tile as tile
from concourse import bacc
from concourse.bass_utils import run_bass_kernel_spmd
from concourse.masks import make_causal_mask, make_identity

B, S, D, H = 8, 2048, 768, 768
N_CORES = 8
P = 128
DT = D // P   # 6 d-tiles
HT = H // P   # 6 h-tiles
ST = S // P   # 16 s-tiles

f32 = mybir.dt.float32
bf16 = mybir.dt.bfloat16
f16 = mybir.dt.float16


def _ceil_div(a, b):
    return (a + b - 1) // b


def build_nc():
    nc = bacc.Bacc(None)

    # all inputs ship pre-tiled from the host in exact SBUF layout
    # ([128 partitions, ...]) so every DMA line is fully contiguous
    xb_d = nc.declare_dram_parameter("xb", [P, DT, S], bf16, isOutput=False)
    xl_d = nc.declare_dram_parameter("xl", [P, DT, S], bf16, isOutput=False)
    x16_d = nc.declare_dram_parameter("x16", [S // 512, P, DT, 512], f16, isOutput=False)
    mb_d = nc.declare_dram_parameter("mb", [P, DT, D], bf16, isOutput=False)
    ml_d = nc.declare_dram_parameter("ml", [P, DT, D], bf16, isOutput=False)
    wv16_d = nc.declare_dram_parameter("wv16", [P, DT, H], f16, isOutput=False)
    out_d = nc.declare_dram_parameter("out", [S, H], f32, isOutput=True)

    with tile.TileContext(nc, pool_alloc_mode="queue") as tc, ExitStack() as ctx:
        persist = ctx.enter_context(tc.tile_pool(name="persist", bufs=1))
        t_b = persist.tile([P, DT, S], bf16)   # 24KB/part each
        t_l = persist.tile([P, DT, S], bf16)
        xb_s = persist.tile([P, DT, S], bf16)
        xl_s = persist.tile([P, DT, S], bf16)
        V = persist.tile([P, ST, H], f16)      # 24KB/part
        ident16 = persist.tile([P, P], f16)
        cmask = persist.tile([P, P], f32)
        make_identity(nc, ident16)
        make_causal_mask(nc, cmask, mask_val=-1e10)

        mpool = tc.alloc_tile_pool(name="p1a_m", bufs=1)
        m_b = mpool.tile([P, DT, D], bf16)
        m_l = mpool.tile([P, DT, D], bf16)

        # ---- phase 1b: V (fp16) — first; its small DMAs start PE fastest -
        with tc.tile_pool(name="p1b_w", bufs=1) as wvpool, \
             tc.tile_pool(name="p1b_x", bufs=4) as xfpool, \
             tc.tile_pool(name="p1b_ps", bufs=4, space="PSUM") as pp:
            wv_s = wvpool.tile([P, DT, H], f16)
            # first matmul group only needs wv[:, :, :384] and x16 chunk0
            # cols [:128] — split those loads so it waits on the minimum
            nc.sync.dma_start(out=wv_s[:, :, :384], in_=wv16_d[:, :, :384])
            nc.sync.dma_start(out=wv_s[:, :, 384:], in_=wv16_d[:, :, 384:])
            bulk_anchor = None
            for sc in range(S // 512):
                xfc = xfpool.tile([P, DT, 512], f16, tag="xfc", name="xfc")
                if sc == 0:
                    nc.sync.dma_start(out=xfc[:, :, :P], in_=x16_d[0][:, :, :P])
                    nc.sync.dma_start(out=xfc[:, :, P:], in_=x16_d[0][:, :, P:])
                else:
                    nc.sync.dma_start(out=xfc, in_=x16_d[sc])
                if sc == 1:
                    # phase-1a/2 bulk loads (8.6MB) stream in behind V
                    # compute on the SWDGE rings; the explicit dep on the
                    # first V-chunk's last copy keeps them from saturating
                    # HBM while the latency-critical first chunks transfer
                    assert bulk_anchor is not None
                    for dst, src in ((xb_s, xb_d), (xl_s, xl_d),
                                     (m_b, mb_d), (m_l, ml_d)):
                        dma = nc.gpsimd.dma_start(out=dst, in_=src[:, :, :])
                        bass_rust.add_dep_helper(
                            dma.ins, bulk_anchor.ins, sync=True,
                            reason="bulk load waits for first V chunk")
                for sti in range(4):
                    off = sti * P
                    for hc in range(2):
                        ps = pp.tile([P, 384], f32, tag="psv", name="psv")
                        for dt_ in range(DT):
                            nc.tensor.matmul(
                                ps,
                                xfc[:, dt_, off:off + P],
                                wv_s[:, dt_, hc * 384:(hc + 1) * 384],
                                start=(dt_ == 0),
                                stop=(dt_ == DT - 1),
                            )
                        cp = nc.any.tensor_copy(
                            V[:, sc * 4 + sti, hc * 384:(hc + 1) * 384], ps)
                        if sc == 0 and sti == 0 and hc == 1:
                            bulk_anchor = cp

        # ---- phase 1a: tT = M^T-stationary x xT (bf16 hi/lo x3) ----------
        with tc.tile_pool(name="p1a_ps", bufs=4, space="PSUM") as pp:
            for sc in range(S // 512):
                for ht in range(HT):
                    ps = pp.tile([P, 512], f32, tag="ps", name="ps")
                    idx = 0
                    for dt_ in range(DT):
                        for wop, xop in ((m_b, xb_s), (m_b, xl_s), (m_l, xb_s)):
                            nc.tensor.matmul(
                                ps,
                                wop[:, dt_, ht * P:(ht + 1) * P],
                                xop[:, dt_, sc * 512:(sc + 1) * 512],
                                start=(idx == 0),
                                stop=(idx == 3 * DT - 1),
                            )
                            idx += 1
                    hi_slice = t_b[:, ht, sc * 512:(sc + 1) * 512]
                    nc.scalar.copy(hi_slice, ps)
                    nc.vector.tensor_sub(
                        t_l[:, ht, sc * 512:(sc + 1) * 512], ps, hi_slice)
        mpool.release()

        # ---- phase 2: attention ------------------------------------------
        with tc.tile_pool(name="p2_strip", bufs=3) as strip_pool, \
             tc.tile_pool(name="p2_exp", bufs=3) as exp_pool, \
             tc.tile_pool(name="p2_expT", bufs=2) as expT_pool, \
             tc.tile_pool(name="p2_stat", bufs=4) as stat_pool, \
             tc.tile_pool(name="p2_out", bufs=2) as out_pool, \
             tc.tile_pool(name="p2_ps_s", bufs=4, space="PSUM") as ps_s_pool, \
             tc.tile_pool(name="p2_ps_t", bufs=2, space="PSUM") as ps_t_pool, \
             tc.tile_pool(name="p2_ps_o", bufs=2, space="PSUM") as ps_o_pool:
            for qt in reversed(range(ST)):
                L = qt + 1
                cols = L * P
                strip = strip_pool.tile([P, S], f32, tag="strip", name="strip")
                for nch in range(_ceil_div(cols, 512)):
                    w = min(512, cols - nch * 512)
                    ps = ps_s_pool.tile([P, 512], f32, tag="ps_s", name="ps_s")
                    idx = 0
                    for dt_ in range(DT):
                        for qop, kop in ((t_b, xb_s), (t_b, xl_s), (t_l, xb_s)):
                            nc.tensor.matmul(
                                ps[:, :w],
                                qop[:, dt_, qt * P:(qt + 1) * P],
                                kop[:, dt_, nch * 512:nch * 512 + w],
                                start=(idx == 0),
                                stop=(idx == 3 * DT - 1),
                            )
                            idx += 1
                    nc.vector.tensor_copy(
                        strip[:, nch * 512:nch * 512 + w], ps[:, :w])
                nc.vector.tensor_add(
                    strip[:, (L - 1) * P:cols],
                    strip[:, (L - 1) * P:cols],
                    cmask,
                )
                nrmax = stat_pool.tile([P, 1], f32, tag="nrmax", name="nrmax")
                nc.vector.tensor_reduce(
                    nrmax, strip[:, :cols],
                    axis=mybir.AxisListType.X, op=mybir.AluOpType.max,
                    negate=True,
                )
                rsum = stat_pool.tile([P, 1], f32, tag="rsum", name="rsum")
                exp16 = exp_pool.tile([P, S], f16, tag="exp16", name="exp16")
                nc.scalar.activation(
                    exp16[:, :cols], strip[:, :cols],
                    mybir.ActivationFunctionType.Exp,
                    bias=nrmax, scale=1.0, accum_out=rsum,
                )
                rinv = stat_pool.tile([P, 1], f32, tag="rinv", name="rinv")
                nc.vector.reciprocal(rinv, rsum)
                expT = expT_pool.tile([P, ST, P], f16, tag="expT", name="expT")
                for j in range(L):
                    pst = ps_t_pool.tile([P, P], f16, tag="ps_t", name="ps_t")
                    nc.tensor.transpose(
                        pst, exp16[:, j * P:(j + 1) * P], ident16)
                    nc.any.tensor_copy(expT[:, j, :], pst)
                out_sb = out_pool.tile([P, H], f32, tag="out_sb", name="out_sb")
                for hc in range(2):
                    pso = ps_o_pool.tile([P, 384], f32, tag="ps_o", name="ps_o")
                    for j in range(L):
                        nc.tensor.matmul(
                            pso,
                            expT[:, j, :],
                            V[:, j, hc * 384:(hc + 1) * 384],
                            start=(j == 0),
                            stop=(j == L - 1),
                        )
                    nc.vector.tensor_scalar_mul(
                        out_sb[:, hc * 384:(hc + 1) * 384], pso, rinv)
                nc.sync.dma_start(
                    out=out_d[qt * P:(qt + 1) * P, :], in_=out_sb)

    nc.finalize()
    return nc


_NC_CACHE = None


def _get_nc():
    global _NC_CACHE
    if _NC_CACHE is None:
        _NC_CACHE = build_nc()
    return _NC_CACHE


def _split_b16(a):
    hi = a.astype(ml_dtypes.bfloat16)
    lo = (a - hi.astype(np.float32)).astype(ml_dtypes.bfloat16)
    return hi, lo


def _tile_rows(a):
    """[D, N] -> [128, D//128, N] (partition-major SBUF layout)."""
    d, n = a.shape
    return np.ascontiguousarray(a.reshape(d // P, P, n).transpose(1, 0, 2))


def make_in_maps(x, Wq, Wk, Wv):
    M = (Wq.astype(np.float64) @ Wk.astype(np.float64).T).astype(np.float32)
    mb, ml = _split_b16(M)
    mb, ml = _tile_rows(mb), _tile_rows(ml)
    wv16 = _tile_rows(Wv.astype(np.float16))
    in_maps = []
    for c in range(N_CORES):
        xT = np.ascontiguousarray(x[c].T)
        xb, xl = _split_b16(xT)
        x16 = np.ascontiguousarray(
            xT.astype(np.float16).reshape(DT, P, S // 512, 512)
            .transpose(2, 1, 0, 3))
        in_maps.append({
            "xb": _tile_rows(xb), "xl": _tile_rows(xl), "x16": x16,
            "mb": mb, "ml": ml, "wv16": wv16,
        })
    return in_maps


def kernel(x, Wq, Wk, Wv):
    x = np.asarray(x, dtype=np.float32)
    Wq = np.asarray(Wq, dtype=np.float32)
    Wk = np.asarray(Wk, dtype=np.float32)
    Wv = np.asarray(Wv, dtype=np.float32)

    nc = _get_nc()
    in_maps = make_in_maps(x, Wq, Wk, Wv)
    res = run_bass_kernel_spmd(nc, in_maps, list(range(N_CORES)))
    out = np.stack([res.results[c]["out"] for c in range(N_CORES)], axis=0)
    return out.astype(np.float32)


# revision 18
# speedup vs baseline: 1.0591x; 1.0591x over previous
"""Causal attention (B=8, S=2048, D=H=768) on 8 trn2 NeuronCores.

Data-parallel over batch: core c computes batch c entirely on-chip, no
collectives.  All matmuls contract over the partition dim.

Key algebraic move: scores = (x Wq)(x Wk)^T = x (Wq Wk^T) x^T, with
M = Wq Wk^T precomputed on host (768x768, ~0.5 GFLOP — negligible).  That
folds the q AND k projections into ONE on-device projection t = x M, and
the scores' k-side operand becomes raw x^T, whose exact bf16 hi/lo splits
ship straight from the host.

Precision scheme (validated vs fp64 in numpy: ~4e-4 rel absmax):
  - t = x M and scores = t x^T use bf16 hi/lo splits with 3-term matmuls
    (hi*hi + hi*lo + lo*hi, fp32 PSUM accumulation) — ~17-bit effective
    mantissa at 3 PE cycles/row.
  - V projection, exp weights, transposes, and attn@V run in fp16
    (11-bit mantissa, 1 cycle/row, fast weight load).
  - softmax stats (rowmax, rowsum, reciprocal) in fp32.

Per-core pipeline:
  phase 1b (first — small DMAs get PE going fastest):
      V[s,h] = x^T-blocks (stationary) x Wv (moving), fp16
  phase 1a: tT = M (stationary) x xT (moving), split to bf16 hi/lo
  phase 2, per 128-row q-tile (descending, so the exposed tail chain is
    the smallest tile): scores strip [q, k<=q]; causal mask on diag block;
    rowmax; exp (ScalarE, bias=-rowmax, accum_out=rowsum) -> fp16;
    PE-transpose exp blocks -> expT [k,q]; out = sum_k expT x V; scale by
    1/rowsum.

Host side: shards x over batch, pre-transposes/splits, computes M,
replicates weights, gathers per-core outputs.
"""

from contextlib import ExitStack

import ml_dtypes
import numpy as np

import bass_rust
import concourse.bass as bass
import concourse.mybir as mybir
import concourse.tile as tile
from concourse import bacc
from concourse.bass_utils import run_bass_kernel_spmd
from concourse.masks import make_causal_mask, make_identity

B, S, D, H = 8, 2048, 768, 768
N_CORES = 8
P = 128
DT = D // P   # 6 d-tiles
HT = H // P   # 6 h-tiles
ST = S // P   # 16 s-tiles

f32 = mybir.dt.float32
bf16 = mybir.dt.bfloat16
f16 = mybir.dt.float16


def _ceil_div(a, b):
    return (a + b - 1) // b


def build_nc():
    nc = bacc.Bacc(None)

    # all inputs ship pre-tiled from the host in exact SBUF layout
    # ([128 partitions, ...]) so every DMA line is fully contiguous
    xb_d = nc.declare_dram_parameter("xb", [P, DT, S], bf16, isOutput=False)
    xl_d = nc.declare_dram_parameter("xl", [P, DT, S], bf16, isOutput=False)
    x16_d = nc.declare_dram_parameter("x16", [S // 512, P, DT, 512], f16, isOutput=False)
    mb_d = nc.declare_dram_parameter("mb", [P, DT, D], bf16, isOutput=False)
    ml_d = nc.declare_dram_parameter("ml", [P, DT, D], bf16, isOutput=False)
    wv16_d = nc.declare_dram_parameter("wv16", [P, DT, H], f16, isOutput=False)
    out_d = nc.declare_dram_parameter("out", [S, H], f32, isOutput=True)

    with tile.TileContext(nc, pool_alloc_mode="queue") as tc, ExitStack() as ctx:
        persist = ctx.enter_context(tc.tile_pool(name="persist", bufs=1))
        t_b = persist.tile([P, DT, S], bf16)   # 24KB/part each
        t_l = persist.tile([P, DT, S], bf16)
        xb_s = persist.tile([P, DT, S], bf16)
        xl_s = persist.tile([P, DT, S], bf16)
        V = persist.tile([P, ST, H], f16)      # 24KB/part
        ident16 = persist.tile([P, P], f16)
        cmask = persist.tile([P, P], f32)
        make_identity(nc, ident16)
        make_causal_mask(nc, cmask, mask_val=-1e10)

        mpool = tc.alloc_tile_pool(name="p1a_m", bufs=1)
        m_b = mpool.tile([P, DT, D], bf16)
        m_l = mpool.tile([P, DT, D], bf16)

        # ---- phase 1b: V (fp16) — first; its small DMAs start PE fastest -
        with tc.tile_pool(name="p1b_w", bufs=1) as wvpool, \
             tc.tile_pool(name="p1b_x", bufs=4) as xfpool, \
             tc.tile_pool(name="p1b_ps", bufs=4, space="PSUM") as pp:
            wv_s = wvpool.tile([P, DT, H], f16)
            nc.sync.dma_start(out=wv_s, in_=wv16_d[:, :, :])
            bulk_anchor = None
            for sc in range(S // 512):
                xfc = xfpool.tile([P, DT, 512], f16, tag="xfc", name="xfc")
                nc.sync.dma_start(out=xfc, in_=x16_d[sc])
                if sc == 1:
                    # phase-1a/2 bulk loads (8.6MB) stream in behind V
                    # compute on the SWDGE rings; the explicit dep on the
                    # first V-chunk's last copy keeps them from saturating
                    # HBM while the latency-critical first chunks transfer
                    assert bulk_anchor is not None
                    for dst, src in ((xb_s, xb_d), (xl_s, xl_d),
                                     (m_b, mb_d), (m_l, ml_d)):
                        dma = nc.gpsimd.dma_start(out=dst, in_=src[:, :, :])
                        bass_rust.add_dep_helper(
                            dma.ins, bulk_anchor.ins, sync=True,
                            reason="bulk load waits for first V chunk")
                for sti in range(4):
                    off = sti * P
                    for hc in range(2):
                        ps = pp.tile([P, 384], f32, tag="psv", name="psv")
                        for dt_ in range(DT):
                            nc.tensor.matmul(
                                ps,
                                xfc[:, dt_, off:off + P],
                                wv_s[:, dt_, hc * 384:(hc + 1) * 384],
                                start=(dt_ == 0),
                                stop=(dt_ == DT - 1),
                            )
                        cp = nc.any.tensor_copy(
                            V[:, sc * 4 + sti, hc * 384:(hc + 1) * 384], ps)
                        if sc == 0 and sti == 0 and hc == 1:
                            bulk_anchor = cp

        # ---- phase 1a: tT = M^T-stationary x xT (bf16 hi/lo x3) ----------
        with tc.tile_pool(name="p1a_ps", bufs=4, space="PSUM") as pp:
            for sc in range(S // 512):
                for ht in range(HT):
                    ps = pp.tile([P, 512], f32, tag="ps", name="ps")
                    idx = 0
                    for dt_ in range(DT):
                        for wop, xop in ((m_b, xb_s), (m_b, xl_s), (m_l, xb_s)):
                            nc.tensor.matmul(
                                ps,
                                wop[:, dt_, ht * P:(ht + 1) * P],
                                xop[:, dt_, sc * 512:(sc + 1) * 512],
                                start=(idx == 0),
                                stop=(idx == 3 * DT - 1),
                            )
                            idx += 1
                    hi_slice = t_b[:, ht, sc * 512:(sc + 1) * 512]
                    nc.scalar.copy(hi_slice, ps)
                    nc.vector.tensor_sub(
                        t_l[:, ht, sc * 512:(sc + 1) * 512], ps, hi_slice)
        mpool.release()

        # ---- phase 2: attention ------------------------------------------
        with tc.tile_pool(name="p2_strip", bufs=3) as strip_pool, \
             tc.tile_pool(name="p2_exp", bufs=3) as exp_pool, \
             tc.tile_pool(name="p2_expT", bufs=2) as expT_pool, \
             tc.tile_pool(name="p2_stat", bufs=4) as stat_pool, \
             tc.tile_pool(name="p2_out", bufs=2) as out_pool, \
             tc.tile_pool(name="p2_ps_s", bufs=4, space="PSUM") as ps_s_pool, \
             tc.tile_pool(name="p2_ps_t", bufs=2, space="PSUM") as ps_t_pool, \
             tc.tile_pool(name="p2_ps_o", bufs=2, space="PSUM") as ps_o_pool:
            for qt in reversed(range(ST)):
                L = qt + 1
                cols = L * P
                strip = strip_pool.tile([P, S], f32, tag="strip", name="strip")
                for nch in range(_ceil_div(cols, 512)):
                    w = min(512, cols - nch * 512)
                    ps = ps_s_pool.tile([P, 512], f32, tag="ps_s", name="ps_s")
                    idx = 0
                    for dt_ in range(DT):
                        for qop, kop in ((t_b, xb_s), (t_b, xl_s), (t_l, xb_s)):
                            nc.tensor.matmul(
                                ps[:, :w],
                                qop[:, dt_, qt * P:(qt + 1) * P],
                                kop[:, dt_, nch * 512:nch * 512 + w],
                                start=(idx == 0),
                                stop=(idx == 3 * DT - 1),
                            )
                            idx += 1
                    nc.vector.tensor_copy(
                        strip[:, nch * 512:nch * 512 + w], ps[:, :w])
                nc.vector.tensor_add(
                    strip[:, (L - 1) * P:cols],
                    strip[:, (L - 1) * P:cols],
                    cmask,
                )
                nrmax = stat_pool.tile([P, 1], f32, tag="nrmax", name="nrmax")
                nc.vector.tensor_reduce(
                    nrmax, strip[:, :cols],
                    axis=mybir.AxisListType.X, op=mybir.AluOpType.max,
                    negate=True,
                )
                rsum = stat_pool.tile([P, 1], f32, tag="rsum", name="rsum")
                exp16 = exp_pool.tile([P, S], f16, tag="exp16", name="exp16")
                nc.scalar.activation(
                    exp16[:, :cols], strip[:, :cols],
                    mybir.ActivationFunctionType.Exp,
                    bias=nrmax, scale=1.0, accum_out=rsum,
                )
                rinv = stat_pool.tile([P, 1], f32, tag="rinv", name="rinv")
                nc.vector.reciprocal(rinv, rsum)
                expT = expT_pool.tile([P, ST, P], f16, tag="expT", name="expT")
                for j in range(L):
                    pst = ps_t_pool.tile([P, P], f16, tag="ps_t", name="ps_t")
                    nc.tensor.transpose(
                        pst, exp16[:, j * P:(j + 1) * P], ident16)
                    nc.any.tensor_copy(expT[:, j, :], pst)
                out_sb = out_pool.tile([P, H], f32, tag="out_sb", name="out_sb")
                for hc in range(2):
                    pso = ps_o_pool.tile([P, 384], f32, tag="ps_o", name="ps_o")
                    for j in range(L):
                        nc.tensor.matmul(
                            pso,
                            expT[:, j, :],
                            V[:, j, hc * 384:(hc + 1) * 384],
                            start=(j == 0),
                            stop=(j == L - 1),
                        )
                    nc.vector.tensor_scalar_mul(
                        out_sb[:, hc * 384:(hc + 1) * 384], pso, rinv)
                nc.sync.dma_start(
                    out=out_d[qt * P:(qt + 1) * P, :], in_=out_sb)

    nc.finalize()
    return nc


_NC_CACHE = None


def _get_nc():
    global _NC_CACHE
    if _NC_CACHE is None:
        _NC_CACHE = build_nc()
    return _NC_CACHE


def _split_b16(a):
    hi = a.astype(ml_dtypes.bfloat16)
    lo = (a - hi.astype(np.float32)).astype(ml_dtypes.bfloat16)
    return hi, lo


def _tile_rows(a):
    """[D, N] -> [128, D//128, N] (partition-major SBUF layout)."""
    d, n = a.shape
    return np.ascontiguousarray(a.reshape(d // P, P, n).transpose(1, 0, 2))


def make_in_maps(x, Wq, Wk, Wv):
    M = (Wq.astype(np.float64) @ Wk.astype(np.float64).T).astype(np.float32)
    mb, ml = _split_b16(M)
    mb, ml = _tile_rows(mb), _tile_rows(ml)
    wv16 = _tile_rows(Wv.astype(np.float16))
    in_maps = []
    for c in range(N_CORES):
        xT = np.ascontiguousarray(x[c].T)
        xb, xl = _split_b16(xT)
        x16 = np.ascontiguousarray(
            xT.astype(np.float16).reshape(DT, P, S // 512, 512)
            .transpose(2, 1, 0, 3))
        in_maps.append({
            "xb": _tile_rows(xb), "xl": _tile_rows(xl), "x16": x16,
            "mb": mb, "ml": ml, "wv16": wv16,
        })
    return in_maps


def kernel(x, Wq, Wk, Wv):
    x = np.asarray(x, dtype=np.float32)
    Wq = np.asarray(Wq, dtype=np.float32)
    Wk = np.asarray(Wk, dtype=np.float32)
    Wv = np.asarray(Wv, dtype=np.float32)

    nc = _get_nc()
    in_maps = make_in_maps(x, Wq, Wk, Wv)
    res = run_bass_kernel_spmd(nc, in_maps, list(range(N_CORES)))
    out = np.stack([res.results[c]["out"] for c in range(N_CORES)], axis=0)
    return out.astype(np.float32)
